# revision 1
# baseline (speedup 1.0000x reference)
"""Trainium2 Bass kernel for nn_EnhancedObj (gnn_message_passing).

Per batch sample (data-parallel over 8 cores, one sample per core):
    ve  = LN(tanh(visual @ W_v + b_v))                  [64, 2048]
    oe  = LN(tanh(obj_flat @ W_o + b_o))                [2304, 2048]
    adj = softmax_n(oe @ ve^T / sqrt(2048))             [2304, 64]
    out = LN(tanh(adj^T @ oe + ve))                     [64, 2048]

All matmuls run in fp16 (fp32 PSUM accumulate) — fp16 streams at the
same 1 col/cycle as bf16 on the TRN2 PE but carries a 10-bit mantissa
(verified vs fp32 reference: maxabs ~3e-3 on a ~1.4-absmax output,
rel-fro ~3.6e-4).  Softmax and all LayerNorm statistics are fp32.

Schedule: ONE fused PE stream.  Phase B starts immediately (chunk 0
paced by W_o slice arrival); the visual branch (A) is emitted between
object chunks 3 and 4, consuming W_v that streamed in behind W_o; the
adjacency (C) and aggregation (D) matmuls interleave into the stream
two chunks at a time, with oe transposes riding the sync HWDGE queue
behind the weight streams.  Softmax uses unnormalized exp weights (logits are O(1)-
bounded, so no max subtraction); the aggregation is rescaled by the
global 1/sum at the end, so nothing serializes behind a full softmax.
PSUM: 3 banks B quarters + 1 bank C + 4 banks (A, then D agg) = 8.

The device kernel assumes the spec's deterministic fills (zero biases,
unit gains).  If non-trivial bias/gain vectors are ever passed, we
fall back to an exact fp32 numpy implementation.
"""

import numpy as np

F16 = np.float16

BS = 8          # batch (== number of cores)
F = 64          # win_len (frames)
OBJ = 36        # objects per frame
D = 2048        # feature dim
N = F * OBJ     # 2304 objects per sample
NCH = N // 128  # 18 object-row chunks
NW = NCH // 2   # 9 two-chunk adjacency windows
KC = D // 128   # 16 contraction chunks
DW = 512        # matmul moving width (one PSUM bank of fp32)
ND = D // DW    # 4 output-column groups
LN_EPS = 1e-5

_BUILD_CACHE = {}


def _f32(x):
    return np.ascontiguousarray(np.asarray(x), dtype=np.float32)


def _klc_layout(w):
    """[D, M] -> [128(kl), KC*M] with element (kl, kc, m) = w[kc*128+kl, m]."""
    d, m = w.shape
    assert d == D
    return w.reshape(KC, 128, m).transpose(1, 0, 2).reshape(128, KC * m)


def _build():
    """Build + compile the SPMD Bass program (trivial-fill fast path)."""
    if "nc" in _BUILD_CACHE:
        return _BUILD_CACHE["nc"]

    import concourse.bacc as bacc
    import concourse.tile as tile
    from concourse import mybir

    f32 = mybir.dt.float32
    f16 = mybir.dt.float16
    AF = mybir.ActivationFunctionType
    AX = mybir.AxisListType
    OP = mybir.AluOpType

    nc = bacc.Bacc("TRN2", target_bir_lowering=False, debug=False, num_devices=BS)

    objT_d = nc.dram_tensor("objT", [NCH, 128, KC * 128], f16, kind="ExternalInput").ap()
    wo_d = nc.dram_tensor("Wo", [128, KC * D], f16, kind="ExternalInput").ap()
    wv_d = nc.dram_tensor("Wv", [128, KC * D], f16, kind="ExternalInput").ap()
    vt_d = nc.dram_tensor("vT", [128, KC * F], f16, kind="ExternalInput").ap()
    out_d = nc.dram_tensor("out", [F, D], f32, kind="ExternalOutput").ap()

    inv_sqrt_d = 1.0 / float(np.sqrt(D))

    # adjacency (C) / aggregation (D) emission points: window w covers
    # object chunks (2w, 2w+1); C(w) needs both transposed + veT (ready
    # after chunk 4); D(w) follows C(w) one chunk later.
    sched = {}
    for w in range(NW):
        c_at = max(2 * w + 3, 5 + (0 if w < 3 else 0)) if w >= 3 else 5 + w
        c_at = min(c_at, NCH - 1) if w < NW - 1 else NCH  # NCH == post-loop
        d_at = c_at + 1
        if c_at < NCH:
            sched.setdefault(c_at, []).append(("C", w))
        if d_at < NCH:
            sched.setdefault(d_at, []).append(("D", w))

    with tile.TileContext(nc) as tc:
        with tc.tile_pool(name="persist", bufs=1) as persist, \
             tc.tile_pool(name="stats", bufs=2) as stats_pool:

            eps128 = persist.tile([128, 1], f32)
            nc.vector.memset(eps128, LN_EPS)

            def layer_norm_to(t_in, rows, out_tile):
                """LN over the free dim of t_in[:rows] -> out_tile (casts)."""
                st = stats_pool.tile([128, ND, nc.vector.BN_STATS_DIM], f32, tag="st")
                for j in range(ND):
                    nc.vector.bn_stats(out=st[:rows, j, :],
                                       in_=t_in[:rows, j * DW:(j + 1) * DW])
                mvr = stats_pool.tile([128, 3], f32, tag="mvr")
                nc.vector.bn_aggr(out=mvr[:rows, 0:2], in_=st[:rows])
                nc.scalar.activation(out=mvr[:rows, 2:3], in_=mvr[:rows, 1:2],
                                     func=AF.Sqrt, bias=eps128[:rows], scale=1.0)
                nc.vector.reciprocal(out=mvr[:rows, 2:3], in_=mvr[:rows, 2:3])
                nc.vector.tensor_scalar(
                    out=out_tile[:rows], in0=t_in[:rows],
                    scalar1=mvr[:rows, 0:1], scalar2=mvr[:rows, 2:3],
                    op0=OP.subtract, op1=OP.mult)

            ve_nat = persist.tile([F, D], f32)          # LN'd visual embedding
            veT = persist.tile([128, KC, F], f16)       # transposed, for adjacency
            oe_nat = persist.tile([128, NCH, D], f16)   # LN'd object embeddings
            psum_w = persist.tile([F, NW + 1], f32)     # per-window exp sums

            with tc.tile_pool(name="wo", bufs=1) as wop, \
                 tc.tile_pool(name="objs", bufs=2) as objp, \
                 tc.tile_pool(name="psB", bufs=3, space="PSUM") as psB, \
                 tc.tile_pool(name="psC", bufs=1, space="PSUM") as psC, \
                 tc.tile_pool(name="ew", bufs=1) as ewp, \
                 tc.tile_pool(name="tmpB", bufs=2) as tmpB:
                wo = wop.tile([128, KC * D], f16)

                # DMA plan: objT loads ride the scalar HWDGE queue; W_o,
                # then W_v, then all transposes stream on the sync queue
                # (W_v's slot-waits resolve before any transpose is due).
                obj_tiles = {}

                def load_objT(nch):
                    t = objp.tile([128, KC, 128], f16, name="objT", tag="objT")
                    nc.scalar.dma_start(out=t, in_=objT_d[nch])
                    obj_tiles[nch] = t

                load_objT(0)
                load_objT(1)
                for kc in range(KC):
                    nc.sync.dma_start(out=wo[:, kc * D:(kc + 1) * D],
                                      in_=wo_d[:, kc * D:(kc + 1) * D])

                win_tiles = {}
                en_tiles = {}
                pending_transpose = []

                def emit_transpose(nch):
                    w = nch // 2
                    if w not in win_tiles:
                        win_tiles[w] = tc_win.tile([128, 2, KC, 128], f16,
                                                   name="winT", tag="winT")
                    nc.sync.dma_start(out=win_tiles[w][:, nch % 2, :, :],
                                      in_=oe_nat[:, nch, :], transpose=True)

                def emit_chunk_B(nch):
                    objT_nc = obj_tiles.pop(nch)
                    if nch + 2 < NCH:
                        load_objT(nch + 2)
                    tB = tmpB.tile([128, D], f16, tag="tB")
                    # quarter-width PSUM tiles (1 bank each, 3 bufs) so each
                    # quarter's tanh overlaps the next quarter's matmuls.
                    for q in range(ND):
                        pq = psB.tile([128, DW], f32, tag="psb")
                        for kc in range(KC):
                            nc.tensor.matmul(
                                pq,
                                lhsT=objT_nc[:, kc, :],
                                rhs=wo[:, kc * D + q * DW: kc * D + (q + 1) * DW],
                                start=(kc == 0), stop=(kc == KC - 1))
                        nc.scalar.activation(out=tB[:, q * DW:(q + 1) * DW],
                                             in_=pq, func=AF.Tanh)
                    layer_norm_to(tB, 128, oe_nat[:, nch, :])

                def emit_window_C(w):
                    """Adjacency + exp for window w (chunks 2w, 2w+1)."""
                    wt = win_tiles.pop(w)
                    padj = psC.tile([F, 256], f32, tag="padj")
                    for kc in range(KC):
                        nc.tensor.matmul(
                            padj,
                            lhsT=veT[:, kc, :],
                            rhs=wt[:, :, kc, :],
                            start=(kc == 0), stop=(kc == KC - 1))
                    # Unnormalized softmax weights: logits are O(1)-bounded
                    # so exp without max-subtraction is safe; accum_out
                    # collects this window's exp-sum for free.
                    ewt = ewp.tile([F, 256], f32, tag="ew")
                    nc.scalar.activation(out=ewt, in_=padj, func=AF.Exp,
                                         scale=inv_sqrt_d,
                                         accum_out=psum_w[:, w:w + 1])
                    e16 = ewp.tile([F, 256], f16, tag="e16")
                    nc.vector.tensor_copy(out=e16, in_=ewt)
                    en = ewp.tile([128, 2, F], f16, tag="en", bufs=2)
                    # [64, 256] -> rows n: [nw, j, f]
                    nc.sync.dma_start(out=en, in_=e16, transpose=True)
                    en_tiles[w] = en

                def emit_window_D(w):
                    """Aggregation matmuls for window w into ps_agg."""
                    en = en_tiles.pop(w)
                    for j in range(2):
                        for dd in range(ND):
                            nc.tensor.matmul(
                                ps_agg[:, dd * DW:(dd + 1) * DW],
                                lhsT=en[:, j, :],
                                rhs=oe_nat[:, 2 * w + j, dd * DW:(dd + 1) * DW],
                                start=(w == 0 and j == 0),
                                stop=(w == NW - 1 and j == 1))

                # ---- object chunks 0-3 (W_o-arrival paced) ------------
                with tc.tile_pool(name="wv", bufs=4) as wvp, \
                     tc.tile_pool(name="vt", bufs=1) as vtp, \
                     tc.tile_pool(name="psA", bufs=1, space="PSUM") as psA, \
                     tc.tile_pool(name="tmpA", bufs=1) as tmpA:
                    vt = vtp.tile([128, KC, F], f16)
                    nc.scalar.dma_start(out=vt, in_=vt_d)

                    # W_v streams behind W_o on the sync queue; phase A's
                    # matmuls (emitted below) consume it at chunk-4 time.
                    wv_slices = []
                    for kc in range(KC):
                        wv_k = wvp.tile([128, D], f16, tag="wvk")
                        nc.sync.dma_start(out=wv_k, in_=wv_d[:, kc * D:(kc + 1) * D])
                        wv_slices.append(wv_k)

                    for nch in range(4):
                        emit_chunk_B(nch)
                        pending_transpose.append(nch)

                    # ---- phase A: visual branch -----------------------
                    ps_ve = psA.tile([F, D], f32)
                    for kc in range(KC):
                        for dd in range(ND):
                            nc.tensor.matmul(
                                ps_ve[:, dd * DW:(dd + 1) * DW],
                                lhsT=vt[:, kc, :],
                                rhs=wv_slices[kc][:, dd * DW:(dd + 1) * DW],
                                start=(kc == 0), stop=(kc == KC - 1))
                    tA = tmpA.tile([F, D], f32)
                    nc.scalar.activation(out=tA, in_=ps_ve, func=AF.Tanh)
                    layer_norm_to(tA, F, ve_nat)
                    ve_bf = tmpB.tile([F, D], f16, tag="tB")
                    nc.vector.tensor_copy(out=ve_bf, in_=ve_nat)
                    # [64, 2048] -> rows d=(kc*128+kl): [kl, kc, f]
                    nc.sync.dma_start(out=veT, in_=ve_bf, transpose=True)

                # ---- object chunks 4-17 with fused C/D ----------------
                with tc.tile_pool(name="win", bufs=3) as tc_win, \
                     tc.tile_pool(name="psD", bufs=1, space="PSUM") as psD:
                    ps_agg = psD.tile([F, D], f32)

                    for nch in range(4, NCH):
                        emit_chunk_B(nch)
                        # drain deferred chunk 0-3 transposes two at a time
                        # behind the current chunk's matmuls
                        for _ in range(min(2, len(pending_transpose))):
                            emit_transpose(pending_transpose.pop(0))
                        emit_transpose(nch)
                        for kind, w in sched.get(nch, []):
                            (emit_window_C if kind == "C" else emit_window_D)(w)
                    # Drain the last two windows: the final window's
                    # adjacency runs chunk-16's half while chunk-17's
                    # transpose is in flight, with D(7) filling the gap.
                    wt = win_tiles.pop(NW - 1)
                    padj = psC.tile([F, 256], f32, tag="padj")
                    for kc in range(KC):
                        nc.tensor.matmul(
                            padj[:, 0:128], lhsT=veT[:, kc, :],
                            rhs=wt[:, 0:1, kc, :],
                            start=(kc == 0), stop=(kc == KC - 1))
                    emit_window_D(NW - 2)
                    for kc in range(KC):
                        nc.tensor.matmul(
                            padj[:, 128:256], lhsT=veT[:, kc, :],
                            rhs=wt[:, 1:2, kc, :],
                            start=(kc == 0), stop=(kc == KC - 1))
                    ewt = ewp.tile([F, 256], f32, tag="ew")
                    nc.scalar.activation(out=ewt, in_=padj, func=AF.Exp,
                                         scale=inv_sqrt_d,
                                         accum_out=psum_w[:, NW - 1:NW])
                    e16 = ewp.tile([F, 256], f16, tag="e16")
                    nc.vector.tensor_copy(out=e16, in_=ewt)
                    en = ewp.tile([128, 2, F], f16, tag="en", bufs=2)
                    nc.sync.dma_start(out=en, in_=e16, transpose=True)
                    en_tiles[NW - 1] = en
                    emit_window_D(NW - 1)

                    # ---- finalize: rescale by 1/sum, residual, LN -----
                    nc.vector.reduce_sum(out=psum_w[:, NW:NW + 1],
                                         in_=psum_w[:, :NW], axis=AX.X)
                    nc.vector.reciprocal(out=psum_w[:, NW:NW + 1],
                                         in_=psum_w[:, NW:NW + 1])
                    tD = tc_win.tile([F, D], f32, tag="winT")
                    nc.vector.scalar_tensor_tensor(
                        out=tD, in0=ps_agg, scalar=psum_w[:, NW:NW + 1],
                        in1=ve_nat, op0=OP.mult, op1=OP.add)
                    nc.scalar.activation(out=tD, in_=tD, func=AF.Tanh)
                    out_f = tc_win.tile([F, D], f32, tag="winT")
                    layer_norm_to(tD, F, out_f)
                    nc.sync.dma_start(out=out_d, in_=out_f)

    nc.compile()
    _BUILD_CACHE["nc"] = nc
    return nc


def _numpy_fallback(inputs):
    """Exact fp32 implementation for non-trivial bias/gain fills."""
    def ln(x, g, b, eps=LN_EPS):
        mu = x.mean(-1, keepdims=True)
        var = x.var(-1, keepdims=True)
        return (x - mu) / np.sqrt(var + eps) * g + b

    vf = _f32(inputs["visual_feats"])
    of = _f32(inputs["obj_feats"])
    W_v, b_v = _f32(inputs["W_v"]), _f32(inputs["b_v"])
    W_o, b_o = _f32(inputs["W_o"]), _f32(inputs["b_o"])
    out = np.zeros((BS, F, D), np.float32)
    for i in range(BS):
        ve = ln(np.tanh(vf[i] @ W_v + b_v), _f32(inputs["ln_v_g"]), _f32(inputs["ln_v_b"]))
        oe = ln(np.tanh(of[i].reshape(N, D) @ W_o + b_o),
                _f32(inputs["ln_o_g"]), _f32(inputs["ln_o_b"]))
        adj = oe @ ve.T / np.sqrt(D)
        adj = np.exp(adj - adj.max(0, keepdims=True))
        adj /= adj.sum(0, keepdims=True)
        out[i] = ln(np.tanh(adj.T @ oe + ve),
                    _f32(inputs["ln_ov_g"]), _f32(inputs["ln_ov_b"]))
    return out


def _prep_core_inputs(visual, obj_flat, shared):
    """Host-side per-sample layout prep. visual [64,2048] f32, obj_flat [2304,2048] f32."""
    m = {
        "objT": np.ascontiguousarray(
            obj_flat.reshape(NCH, 128, KC, 128).transpose(0, 3, 2, 1)
        ).astype(F16).reshape(NCH, 128, KC * 128),
        "vT": np.ascontiguousarray(
            _klc_layout(np.ascontiguousarray(visual.T))).astype(F16),
    }
    m.update(shared)
    return m


def run_kernel(inputs, trace=False):
    """Returns (out [8, 64, 2048] fp32, exec_time_ns or None)."""
    from concourse import bass_utils

    vecs = {k: _f32(inputs[k]) for k in
            ["b_v", "b_o", "ln_v_b", "ln_o_b", "ln_ov_b"]}
    gains = {k: _f32(inputs[k]) for k in ["ln_v_g", "ln_o_g", "ln_ov_g"]}
    trivial = (all(np.all(v == 0) for v in vecs.values())
               and all(np.all(g == 1) for g in gains.values()))
    if not trivial:
        return _numpy_fallback(inputs), None

    visual = _f32(inputs["visual_feats"])            # [8, 64, 2048]
    obj = _f32(inputs["obj_feats"])                  # [8, 64, 36, 2048]
    W_v = _f32(inputs["W_v"])
    W_o = _f32(inputs["W_o"])

    nc = _build()

    shared = {
        "Wo": np.ascontiguousarray(_klc_layout(W_o)).astype(F16),
        "Wv": np.ascontiguousarray(_klc_layout(W_v)).astype(F16),
    }
    in_maps = [
        _prep_core_inputs(visual[c], obj[c].reshape(N, D), shared)
        for c in range(BS)
    ]

    res = bass_utils.run_bass_kernel_spmd(
        nc, in_maps, core_ids=list(range(BS)), trace=trace)
    out = np.stack([res.results[c]["out"] for c in range(BS)], axis=0)
    return out.astype(np.float32), res.exec_time_ns


def kernel(**inputs):
    out, _ = run_kernel(inputs, trace=False)
    return out



# revision 8
# speedup vs baseline: 1.4704x; 1.4704x over previous
"""Trainium2 Bass kernel for nn_EnhancedObj (gnn_message_passing).

Per batch sample (data-parallel over 8 cores, one sample per core):
    ve  = LN(tanh(visual @ W_v + b_v))                  [64, 2048]
    oe  = LN(tanh(obj_flat @ W_o + b_o))                [2304, 2048]
    adj = softmax_n(oe @ ve^T / sqrt(2048))             [2304, 64]
    out = LN(tanh(adj^T @ oe + ve))                     [64, 2048]

The dominant object-branch GEMM (B) runs in fp8e4 with
perf_mode=DoubleRow (K=256 per instruction, 2 MACs/cell/cycle, ~1.4x
measured over 16-bit at N=512); W_o is pre-scaled by 32 into e4m3's
normal range and the scale is undone inside the tanh.  The
precision-critical visual branch (A: its error passes 1:1 into the
output residual) plus the adjacency (C) and aggregation (D) stay fp16
(fp32 PSUM accumulate).  End-to-end rel-fro vs fp32 reference ~1.8e-3
(fp8 per-element error on oe is ~4% but averages out over 2304
objects in the softmax aggregation).  Softmax and all LayerNorm
statistics are fp32.

Schedule: ONE fused PE stream.  Phase B starts immediately (chunk 0
paced by W_o slice arrival); the visual branch (A) is emitted between
object chunks 3 and 4, consuming W_v that streamed in behind W_o; the
adjacency (C) and aggregation (D) matmuls interleave into the stream
two chunks at a time, with oe transposes riding the sync HWDGE queue
behind the weight streams.  Softmax uses unnormalized exp weights (logits are O(1)-
bounded, so no max subtraction); the aggregation is rescaled by the
global 1/sum at the end, so nothing serializes behind a full softmax.
PSUM: 3 banks B quarters + 1 bank C + 4 banks (A, then D agg) = 8.

The device kernel assumes the spec's deterministic fills (zero biases,
unit gains).  If non-trivial bias/gain vectors are ever passed, we
fall back to an exact fp32 numpy implementation.
"""

import numpy as np
import ml_dtypes

F16 = np.float16
F8 = ml_dtypes.float8_e4m3   # TRN fp8e4 (max +-240, IEEE-style)

BS = 8          # batch (== number of cores)
F = 64          # win_len (frames)
OBJ = 36        # objects per frame
D = 2048        # feature dim
N = F * OBJ     # 2304 objects per sample
NCH = N // 128  # 18 object-row chunks
NW = NCH // 2   # 9 two-chunk adjacency windows
KC = D // 128   # 16 contraction chunks
KC2 = KC // 2   # 8 double-row (K=256) contraction chunks
DW = 512        # matmul moving width (one PSUM bank of fp32)
ND = D // DW    # 4 output-column groups
LN_EPS = 1e-5
WO_SCALE = 32.0  # pre-scale W_o into e4m3's normal range (undone in tanh)

_BUILD_CACHE = {}


def _f32(x):
    return np.ascontiguousarray(np.asarray(x), dtype=np.float32)


def _klc_layout(w):
    """[D, M] -> [128(kl), KC*M] with element (kl, kc, m) = w[kc*128+kl, m]."""
    d, m = w.shape
    assert d == D
    return w.reshape(KC, 128, m).transpose(1, 0, 2).reshape(128, KC * m)


def _build():
    """Build + compile the SPMD Bass program (trivial-fill fast path)."""
    if "nc" in _BUILD_CACHE:
        return _BUILD_CACHE["nc"]

    import concourse.bacc as bacc
    import concourse.tile as tile
    from concourse import mybir

    f32 = mybir.dt.float32
    f16 = mybir.dt.float16
    f8 = mybir.dt.float8e4
    AF = mybir.ActivationFunctionType
    AX = mybir.AxisListType
    OP = mybir.AluOpType
    DR = mybir.MatmulPerfMode.DoubleRow

    nc = bacc.Bacc("TRN2", target_bir_lowering=False, debug=False, num_devices=BS)

    objT_d = nc.dram_tensor("objT", [NCH, 128, KC2 * 2 * 128], f8, kind="ExternalInput").ap()
    wo_d = nc.dram_tensor("Wo", [128, KC2 * 2 * D], f8, kind="ExternalInput").ap()
    wv_d = nc.dram_tensor("Wv", [128, KC * D], f16, kind="ExternalInput").ap()
    vt_d = nc.dram_tensor("vT", [128, KC * F], f16, kind="ExternalInput").ap()
    out_d = nc.dram_tensor("out", [F, D], f32, kind="ExternalOutput").ap()

    inv_sqrt_d = 1.0 / float(np.sqrt(D))

    # adjacency (C) / aggregation (D) emission points: window w covers
    # object chunks (2w, 2w+1); C(w) needs both transposed + veT (ready
    # after chunk 4); D(w) follows C(w) one chunk later.
    sched = {}
    for w in range(NW):
        c_at = max(2 * w + 3, 5 + (0 if w < 3 else 0)) if w >= 3 else 5 + w
        c_at = min(c_at, NCH - 1) if w < NW - 1 else NCH  # NCH == post-loop
        d_at = c_at + 1
        if c_at < NCH:
            sched.setdefault(c_at, []).append(("C", w))
        if d_at < NCH:
            sched.setdefault(d_at, []).append(("D", w))

    with tile.TileContext(nc) as tc:
        with tc.tile_pool(name="persist", bufs=1) as persist, \
             tc.tile_pool(name="stats", bufs=2) as stats_pool:

            eps128 = persist.tile([128, 1], f32)
            nc.vector.memset(eps128, LN_EPS)

            def layer_norm_to(t_in, rows, out_tile):
                """LN over the free dim of t_in[:rows] -> out_tile (casts)."""
                st = stats_pool.tile([128, ND, nc.vector.BN_STATS_DIM], f32, tag="st")
                for j in range(ND):
                    nc.vector.bn_stats(out=st[:rows, j, :],
                                       in_=t_in[:rows, j * DW:(j + 1) * DW])
                mvr = stats_pool.tile([128, 3], f32, tag="mvr")
                nc.vector.bn_aggr(out=mvr[:rows, 0:2], in_=st[:rows])
                nc.scalar.activation(out=mvr[:rows, 2:3], in_=mvr[:rows, 1:2],
                                     func=AF.Sqrt, bias=eps128[:rows], scale=1.0)
                nc.vector.reciprocal(out=mvr[:rows, 2:3], in_=mvr[:rows, 2:3])
                nc.vector.tensor_scalar(
                    out=out_tile[:rows], in0=t_in[:rows],
                    scalar1=mvr[:rows, 0:1], scalar2=mvr[:rows, 2:3],
                    op0=OP.subtract, op1=OP.mult)

            ve_nat = persist.tile([F, D], f32)          # LN'd visual embedding
            veT = persist.tile([128, KC, F], f16)       # transposed, for adjacency
            oe_nat = persist.tile([128, NCH, D], f16)   # LN'd object embeddings
            psum_w = persist.tile([F, NW + 1], f32)     # per-window exp sums

            with tc.tile_pool(name="wo", bufs=1) as wop, \
                 tc.tile_pool(name="objs", bufs=2) as objp, \
                 tc.tile_pool(name="psB", bufs=3, space="PSUM") as psB, \
                 tc.tile_pool(name="psC", bufs=1, space="PSUM") as psC, \
                 tc.tile_pool(name="ew", bufs=1) as ewp, \
                 tc.tile_pool(name="tmpB", bufs=2) as tmpB:
                wo = wop.tile([128, KC2, 2, D], f8)

                # DMA plan: objT loads ride the scalar HWDGE queue; W_o,
                # then W_v, then all transposes stream on the sync queue
                # (W_v's slot-waits resolve before any transpose is due).
                obj_tiles = {}

                def load_objT(nch):
                    t = objp.tile([128, KC2, 2, 128], f8, name="objT", tag="objT")
                    nc.scalar.dma_start(out=t, in_=objT_d[nch])
                    obj_tiles[nch] = t

                load_objT(0)
                load_objT(1)
                for kc2 in range(KC2):
                    nc.sync.dma_start(out=wo[:, kc2],
                                      in_=wo_d[:, kc2 * 2 * D:(kc2 + 1) * 2 * D])

                win_tiles = {}
                en_tiles = {}
                pending_transpose = []

                def emit_transpose(nch):
                    w = nch // 2
                    if w not in win_tiles:
                        win_tiles[w] = tc_win.tile([128, 2, KC, 128], f16,
                                                   name="winT", tag="winT")
                    nc.sync.dma_start(out=win_tiles[w][:, nch % 2, :, :],
                                      in_=oe_nat[:, nch, :], transpose=True)

                def emit_chunk_B(nch):
                    objT_nc = obj_tiles.pop(nch)
                    if nch + 2 < NCH:
                        load_objT(nch + 2)
                    tB = tmpB.tile([128, D], f16, tag="tB")
                    # quarter-width PSUM tiles (1 bank each, 3 bufs) so each
                    # quarter's tanh overlaps the next quarter's matmuls.
                    # fp8 DoubleRow: K=256 per instruction, 2 MACs/cell/cycle.
                    for q in range(ND):
                        pq = psB.tile([128, DW], f32, tag="psb")
                        for kc2 in range(KC2):
                            nc.tensor.matmul(
                                pq,
                                lhsT=objT_nc[:, kc2],
                                rhs=wo[:, kc2, :, q * DW:(q + 1) * DW],
                                start=(kc2 == 0), stop=(kc2 == KC2 - 1),
                                perf_mode=DR)
                        nc.scalar.activation(out=tB[:, q * DW:(q + 1) * DW],
                                             in_=pq, func=AF.Tanh,
                                             scale=1.0 / WO_SCALE)
                    layer_norm_to(tB, 128, oe_nat[:, nch, :])

                def emit_window_C(w):
                    """Adjacency + exp for window w (chunks 2w, 2w+1)."""
                    wt = win_tiles.pop(w)
                    padj = psC.tile([F, 256], f32, tag="padj")
                    for kc in range(KC):
                        nc.tensor.matmul(
                            padj,
                            lhsT=veT[:, kc, :],
                            rhs=wt[:, :, kc, :],
                            start=(kc == 0), stop=(kc == KC - 1))
                    # Unnormalized softmax weights: logits are O(1)-bounded
                    # so exp without max-subtraction is safe; accum_out
                    # collects this window's exp-sum for free.
                    ewt = ewp.tile([F, 256], f32, tag="ew")
                    nc.scalar.activation(out=ewt, in_=padj, func=AF.Exp,
                                         scale=inv_sqrt_d,
                                         accum_out=psum_w[:, w:w + 1])
                    e16 = ewp.tile([F, 256], f16, tag="e16")
                    nc.vector.tensor_copy(out=e16, in_=ewt)
                    en = ewp.tile([128, 2, F], f16, tag="en", bufs=2)
                    # [64, 256] -> rows n: [nw, j, f]
                    nc.sync.dma_start(out=en, in_=e16, transpose=True)
                    en_tiles[w] = en

                def emit_window_D(w):
                    """Aggregation matmuls for window w into ps_agg."""
                    en = en_tiles.pop(w)
                    for j in range(2):
                        for dd in range(ND):
                            nc.tensor.matmul(
                                ps_agg[:, dd * DW:(dd + 1) * DW],
                                lhsT=en[:, j, :],
                                rhs=oe_nat[:, 2 * w + j, dd * DW:(dd + 1) * DW],
                                start=(w == 0 and j == 0),
                                stop=(w == NW - 1 and j == 1))

                # ---- object chunks 0-3 (W_o-arrival paced) ------------
                with tc.tile_pool(name="wv", bufs=4) as wvp, \
                     tc.tile_pool(name="vt", bufs=1) as vtp, \
                     tc.tile_pool(name="psA", bufs=1, space="PSUM") as psA, \
                     tc.tile_pool(name="tmpA", bufs=1) as tmpA:
                    vt = vtp.tile([128, KC, F], f16)
                    nc.scalar.dma_start(out=vt, in_=vt_d)

                    # W_v streams behind W_o on the sync queue; phase A's
                    # matmuls (emitted below) consume it at chunk-4 time.
                    wv_slices = []
                    for kc in range(KC):
                        wv_k = wvp.tile([128, D], f16, tag="wvk")
                        nc.sync.dma_start(out=wv_k, in_=wv_d[:, kc * D:(kc + 1) * D])
                        wv_slices.append(wv_k)

                    for nch in range(4):
                        emit_chunk_B(nch)
                        pending_transpose.append(nch)

                    # ---- phase A: visual branch -----------------------
                    ps_ve = psA.tile([F, D], f32)
                    for kc in range(KC):
                        for dd in range(ND):
                            nc.tensor.matmul(
                                ps_ve[:, dd * DW:(dd + 1) * DW],
                                lhsT=vt[:, kc, :],
                                rhs=wv_slices[kc][:, dd * DW:(dd + 1) * DW],
                                start=(kc == 0), stop=(kc == KC - 1))
                    tA = tmpA.tile([F, D], f32)
                    nc.scalar.activation(out=tA, in_=ps_ve, func=AF.Tanh)
                    layer_norm_to(tA, F, ve_nat)
                    ve_bf = tmpB.tile([F, D], f16, tag="tB")
                    nc.vector.tensor_copy(out=ve_bf, in_=ve_nat)
                    # [64, 2048] -> rows d=(kc*128+kl): [kl, kc, f]
                    nc.sync.dma_start(out=veT, in_=ve_bf, transpose=True)

                # ---- object chunks 4-17 with fused C/D ----------------
                with tc.tile_pool(name="win", bufs=3) as tc_win, \
                     tc.tile_pool(name="psD", bufs=1, space="PSUM") as psD:
                    ps_agg = psD.tile([F, D], f32)

                    for nch in range(4, NCH):
                        emit_chunk_B(nch)
                        # drain deferred chunk 0-3 transposes two at a time
                        # behind the current chunk's matmuls
                        for _ in range(min(2, len(pending_transpose))):
                            emit_transpose(pending_transpose.pop(0))
                        emit_transpose(nch)
                        for kind, w in sched.get(nch, []):
                            (emit_window_C if kind == "C" else emit_window_D)(w)
                    # Drain the last two windows: the final window's
                    # adjacency runs chunk-16's half while chunk-17's
                    # transpose is in flight, with D(7) filling the gap.
                    wt = win_tiles.pop(NW - 1)
                    padj = psC.tile([F, 256], f32, tag="padj")
                    for kc in range(KC):
                        nc.tensor.matmul(
                            padj[:, 0:128], lhsT=veT[:, kc, :],
                            rhs=wt[:, 0:1, kc, :],
                            start=(kc == 0), stop=(kc == KC - 1))
                    emit_window_D(NW - 2)
                    for kc in range(KC):
                        nc.tensor.matmul(
                            padj[:, 128:256], lhsT=veT[:, kc, :],
                            rhs=wt[:, 1:2, kc, :],
                            start=(kc == 0), stop=(kc == KC - 1))
                    ewt = ewp.tile([F, 256], f32, tag="ew")
                    nc.scalar.activation(out=ewt, in_=padj, func=AF.Exp,
                                         scale=inv_sqrt_d,
                                         accum_out=psum_w[:, NW - 1:NW])
                    e16 = ewp.tile([F, 256], f16, tag="e16")
                    nc.vector.tensor_copy(out=e16, in_=ewt)
                    en = ewp.tile([128, 2, F], f16, tag="en", bufs=2)
                    nc.sync.dma_start(out=en, in_=e16, transpose=True)
                    en_tiles[NW - 1] = en
                    emit_window_D(NW - 1)

                    # ---- finalize: rescale by 1/sum, residual, LN -----
                    nc.vector.reduce_sum(out=psum_w[:, NW:NW + 1],
                                         in_=psum_w[:, :NW], axis=AX.X)
                    nc.vector.reciprocal(out=psum_w[:, NW:NW + 1],
                                         in_=psum_w[:, NW:NW + 1])
                    tD = tc_win.tile([F, D], f32, tag="winT")
                    nc.vector.scalar_tensor_tensor(
                        out=tD, in0=ps_agg, scalar=psum_w[:, NW:NW + 1],
                        in1=ve_nat, op0=OP.mult, op1=OP.add)
                    nc.scalar.activation(out=tD, in_=tD, func=AF.Tanh)
                    out_f = tc_win.tile([F, D], f32, tag="winT")
                    layer_norm_to(tD, F, out_f)
                    nc.sync.dma_start(out=out_d, in_=out_f)

    nc.compile()
    _BUILD_CACHE["nc"] = nc
    return nc


def _numpy_fallback(inputs):
    """Exact fp32 implementation for non-trivial bias/gain fills."""
    def ln(x, g, b, eps=LN_EPS):
        mu = x.mean(-1, keepdims=True)
        var = x.var(-1, keepdims=True)
        return (x - mu) / np.sqrt(var + eps) * g + b

    vf = _f32(inputs["visual_feats"])
    of = _f32(inputs["obj_feats"])
    W_v, b_v = _f32(inputs["W_v"]), _f32(inputs["b_v"])
    W_o, b_o = _f32(inputs["W_o"]), _f32(inputs["b_o"])
    out = np.zeros((BS, F, D), np.float32)
    for i in range(BS):
        ve = ln(np.tanh(vf[i] @ W_v + b_v), _f32(inputs["ln_v_g"]), _f32(inputs["ln_v_b"]))
        oe = ln(np.tanh(of[i].reshape(N, D) @ W_o + b_o),
                _f32(inputs["ln_o_g"]), _f32(inputs["ln_o_b"]))
        adj = oe @ ve.T / np.sqrt(D)
        adj = np.exp(adj - adj.max(0, keepdims=True))
        adj /= adj.sum(0, keepdims=True)
        out[i] = ln(np.tanh(adj.T @ oe + ve),
                    _f32(inputs["ln_ov_g"]), _f32(inputs["ln_ov_b"]))
    return out


def _prep_core_inputs(visual, obj_flat, shared):
    """Host-side per-sample layout prep. visual [64,2048] f32, obj_flat [2304,2048] f32."""
    # objT8[nch, ki, kc2, ko, m] = x[nch*128 + m, kc2*256 + ko*128 + ki]
    m = {
        "objT": np.ascontiguousarray(
            obj_flat.reshape(NCH, 128, KC2, 2, 128).transpose(0, 4, 2, 3, 1)
        ).astype(F8).reshape(NCH, 128, KC2 * 2 * 128),
        "vT": np.ascontiguousarray(
            _klc_layout(np.ascontiguousarray(visual.T))).astype(F16),
    }
    m.update(shared)
    return m


def run_kernel(inputs, trace=False):
    """Returns (out [8, 64, 2048] fp32, exec_time_ns or None)."""
    from concourse import bass_utils

    vecs = {k: _f32(inputs[k]) for k in
            ["b_v", "b_o", "ln_v_b", "ln_o_b", "ln_ov_b"]}
    gains = {k: _f32(inputs[k]) for k in ["ln_v_g", "ln_o_g", "ln_ov_g"]}
    trivial = (all(np.all(v == 0) for v in vecs.values())
               and all(np.all(g == 1) for g in gains.values()))
    if not trivial:
        return _numpy_fallback(inputs), None

    visual = _f32(inputs["visual_feats"])            # [8, 64, 2048]
    obj = _f32(inputs["obj_feats"])                  # [8, 64, 36, 2048]
    W_v = _f32(inputs["W_v"])
    W_o = _f32(inputs["W_o"])

    nc = _build()

    # Wo8[ki, kc2, ko, n] = WO_SCALE * W_o[kc2*256 + ko*128 + ki, n]
    shared = {
        "Wo": np.ascontiguousarray(
            (WO_SCALE * W_o).reshape(KC2, 2, 128, D).transpose(2, 0, 1, 3)
        ).astype(F8).reshape(128, KC2 * 2 * D),
        "Wv": np.ascontiguousarray(_klc_layout(W_v)).astype(F16),
    }
    in_maps = [
        _prep_core_inputs(visual[c], obj[c].reshape(N, D), shared)
        for c in range(BS)
    ]

    res = bass_utils.run_bass_kernel_spmd(
        nc, in_maps, core_ids=list(range(BS)), trace=trace)
    out = np.stack([res.results[c]["out"] for c in range(BS)], axis=0)
    return out.astype(np.float32), res.exec_time_ns


def kernel(**inputs):
    out, _ = run_kernel(inputs, trace=False)
    return out

